# revision 5
# baseline (speedup 1.0000x reference)
"""Trainium2 Bass kernel for nn_ConvDY2d (dynamic-weight 3x3 conv, CondConv-style).

Reference computation (B=16, C=O=256, H=W=64, K=4 mixing kernels):
  attn  = softmax(MLP(global_avg_pool(x)) / 30)            # [B, 4]
  w_mix = einsum('bk,koihw->boihw', attn, w_dyn)           # per-sample 3x3 conv kernel
  out[b] = conv2d(x[b], w_mix[b], padding=1)

Strategy: data-parallel over batch, 2 samples per NeuronCore across 8 cores.
Per core, the conv is an implicit GEMM: for each (out-channel block, 8-row
group) a [128, 512] PSUM tile accumulates 18 bf16 matmuls (2 c-blocks x
9 taps) whose rhs are contiguous 512-element slices of a row-padded input
image ([128c, 4226]).  Column wrap-around at row edges is fixed up by
subtracting border corrections computed from compacted border-column gathers.

v2 startup redesign: x[0] is loaded first at full aggregate DMA bandwidth
(8 descriptors with 2KB/partition runs over all 3 queues, weights strictly
after), pooling is split across DVE (c-block 0) and the scalar engine's
activation accumulator (c-block 1) so it pipelines with chunk arrival, and
every engine's instruction stream is a single dep-chain in emission order so
the attention chain (exp on scalar) can never sit behind queue-blocked DMA
issues.  Mixing weights are packed host-side into bf16 tensors wcrit (tap 0)
and wrest1/2 (taps 1-4 / 5-8) so each loads as one large descriptor and the
3-pass DVE mixing runs in 2x 16-bit mode.  Border fixes run on gpsimd.
"""

import sys

if "/opt/trn_rl_repo" not in sys.path:
    sys.path.insert(0, "/opt/trn_rl_repo")

import numpy as np

B, C, H, W = 16, 256, 64, 64
O, K, KS = 256, 4, 3
MID = C // 4
INV_DELTA = 1.0 / 30.0
NCORES = 8
NB = B // NCORES            # samples per core
NPOS = KS * KS              # 9 taps
FPAD = 1 + 66 * W + 1       # padded image free size: 4226
ROW0 = 65                   # flat offset of input row 0 (= 1 + 1*64)

_CACHE = {}


def _build_nc():
    import concourse.bacc as bacc
    import concourse.tile as tile
    from concourse import mybir
    from concourse.tile_rust import add_dep_helper

    f32 = mybir.dt.float32
    AX = mybir.AxisListType
    ALU = mybir.AluOpType
    ACTF = mybir.ActivationFunctionType

    nc = bacc.Bacc(target_bir_lowering=False, debug=False)

    bf16 = mybir.dt.bfloat16

    x_d = nc.dram_tensor("x", [NB, C, H, W], bf16, kind="ExternalInput").ap()
    # packed mixing sources, all bf16:
    #   wfirst: taps0-4 [C, 4*5*O]  = [wbar_t04 | wd1_t04 | wd2_t04 | wd3_t04]
    #   wrest2: taps5-8 [C, 4*4*O]
    wfirst_d = nc.dram_tensor("wfirst", [C, 20 * O], bf16, kind="ExternalInput").ap()
    wrest2_d = nc.dram_tensor("wrest2", [C, 16 * O], bf16, kind="ExternalInput").ap()
    fc1wT_d = nc.dram_tensor("fc1wT", [C, MID], bf16, kind="ExternalInput").ap()
    fc1b_d = nc.dram_tensor("fc1b", [MID, 1], f32, kind="ExternalInput").ap()
    fc2aug_d = nc.dram_tensor("fc2aug", [MID + 1, K], bf16, kind="ExternalInput").ap()
    out_d = nc.dram_tensor("out", [NB, O, H, W], bf16, kind="ExternalOutput").ap()

    # one total-order dep chain per engine (DMA issues + compute together)
    chains = {}

    def chained(key, ins):
        prev = chains.get(key)
        if prev is not None:
            add_dep_helper(ins.ins, prev.ins, sync=False, reason=f"{key} order")
        chains[key] = ins
        return ins

    with tile.TileContext(nc) as tc:
        with (
            tc.tile_pool(name="consts", bufs=1) as constp,
            tc.tile_pool(name="wsrc", bufs=1) as wsrcp,
            tc.tile_pool(name="wmix", bufs=1) as wmixp,
            tc.tile_pool(name="xpad", bufs=1) as xpadp,
            tc.tile_pool(name="osb", bufs=8) as osbp,
            tc.tile_pool(name="convps", bufs=5, space="PSUM") as convps,
            tc.tile_pool(name="corrps", bufs=2, space="PSUM") as corrps,
            tc.tile_pool(name="smallps", bufs=1, space="PSUM") as smallps,
        ):
            def qdma(qname, dst, src):
                eng = {"sync": nc.sync, "scalar": nc.scalar, "gpsimd": nc.gpsimd}[qname]
                return chained(qname, eng.dma_start(dst, src))

            def vop(f, *a, **kw):
                return chained("dve", f(*a, **kw))

            def sop(f, *a, **kw):
                return chained("scalar", f(*a, **kw))

            def gop(f, *a, **kw):
                return chained("gpsimd", f(*a, **kw))

            # ---------------- xpad tiles + pad memsets (gpsimd) --------------------
            xpad = [[None, None] for _ in range(NB)]
            for cb in range(2):
                t = xpadp.tile([128, FPAD], bf16, tag=f"xpad0{cb}", name=f"xpad0{cb}")
                gop(nc.gpsimd.memset, t[:, 0:ROW0], 0.0)
                gop(nc.gpsimd.memset, t[:, ROW0 + H * W : FPAD], 0.0)
                xpad[0][cb] = t

            # ---------------- x[0]: 2 descriptors on the 2 HWDGE queues -----------
            # (the gpsimd SWDGE queue only sustains ~75 GB/s; sync/scalar HWDGE
            # reach ~200 GB/s but only with 8KB per-partition runs — splitting
            # into 4KB runs halves the per-queue rate, so one big descriptor
            # per channel block wins even though pooling then can't pipeline)
            x0_dma = [None, None]
            for cb, qn in ((0, "sync"), (1, "scalar")):
                x0_dma[cb] = qdma(
                    qn,
                    xpad[0][cb][:, ROW0 : ROW0 + H * W],
                    x_d[0, cb * 128 : (cb + 1) * 128, :, :]
                    .rearrange("c h w -> c (h w)"),
                )

            # ---------------- weight + const loads (behind x0 on each queue) -------
            wfirst_sb = [None, None]
            wrest2_sb = [None, None]
            for cb in range(2):
                wfirst_sb[cb] = wsrcp.tile([128, 20 * O], bf16, tag=f"wf{cb}", name=f"wfirst{cb}")
                wrest2_sb[cb] = wsrcp.tile([128, 16 * O], bf16, tag=f"wr{cb}", name=f"wrest2{cb}")
            rows = lambda cb: slice(cb * 128, (cb + 1) * 128)
            # gpsimd queue: small consts + the latest-needed weight tensor
            fc1wT_sb = constp.tile([128, 2 * MID], bf16, tag="fc1w", name="fc1wT_sb")
            for cb in range(2):
                qdma("gpsimd", fc1wT_sb[:, cb * MID : (cb + 1) * MID],
                     fc1wT_d[rows(cb), :])
            fc1b_sb = constp.tile([MID, 1], f32, tag="fc1b", name="fc1b_sb")
            qdma("gpsimd", fc1b_sb, fc1b_d)
            fc2aug_sb = constp.tile([MID + 1, K], bf16, tag="fc2", name="fc2aug_sb")
            qdma("gpsimd", fc2aug_sb, fc2aug_d)
            # Big weight tensors.  Descriptors outstanding on a queue share
            # its bandwidth round-robin (NOT FIFO), so big transfers are
            # paced into phases: each phase's issues wait (sync deps) for the
            # previous phase's DMAs to complete.  Phase pairs ride the two
            # HWDGE queues concurrently at the ~360 GB/s HBM cap.
            def gate(ins, *prevs):
                for p in prevs:
                    add_dep_helper(ins.ins, p.ins, sync=True, reason="dma pacing")
                return ins

            wf_dma = [None, None]
            # own-queue serial chains: gate each transfer only on its queue's
            # predecessor (round-robin sharing is intra-queue; cross-queue
            # HBM contention is mild) so every stream starts ASAP
            wf_dma[0] = gate(qdma("sync", wfirst_sb[0], wfirst_d[rows(0), :]),
                             x0_dma[0])
            # wf_dma[1] (scalar queue) is issued from inside pool_sample(0),
            # between the two accumulates, so neither the pool chain nor
            # x0-cb1's bandwidth is disturbed
            wr_dma = [None, None]

            ones_sb = constp.tile([1, 128], bf16, tag="ones", name="ones_sb")
            vop(nc.vector.memset, ones_sb, 1.0)

            # pooling scratch (activation main output, discarded)
            pscr = constp.tile([128, 2048], bf16, tag="pscr", name="pool_scratch")
            tscr = constp.tile([128, 3072], bf16, tag="tscr", name="tree_scratch")

            # ---------------- pooling: DVE does cb0, scalar-accum does cb1 ---------
            pooled_all = [[None, None] for _ in range(NB)]

            def pool_sample(b):
                # cb0: bf16 TT-add tree on DVE (2x mode) + small fp32 reduce;
                # cb1: scalar accumulates the low half while DVE trees the
                # high half -- pooled1 lands ~2.5us after cb1 arrives.
                i0 = xpad[b][0][:, ROW0 : ROW0 + H * W]
                i1 = xpad[b][1][:, ROW0 : ROW0 + H * W]
                t0 = tscr[:, 0:2048]
                vop(nc.vector.tensor_add, t0, i0[:, 0:2048], i0[:, 2048:4096])
                vop(nc.vector.tensor_add, t0[:, 0:1024], t0[:, 0:1024], t0[:, 1024:2048])
                vop(nc.vector.tensor_add, t0[:, 0:512], t0[:, 0:512], t0[:, 512:1024])
                pt0 = constp.tile([128, 1], f32, tag=f"pt{b}0", name=f"ptmp{b}0")
                vop(nc.vector.reduce_sum, pt0, t0[:, 0:512], AX.X)
                p0 = constp.tile([128, 1], bf16, tag=f"pool{b}0", name=f"pooled{b}0")
                vop(nc.vector.tensor_scalar_add, p0, pt0, 0.0)
                pooled_all[b][0] = p0
                if b == 0:
                    # wfirst-cb1 issue, gated on x0-cb1 (own queue) only
                    wf_dma[1] = gate(
                        qdma("scalar", wfirst_sb[1], wfirst_d[rows(1), :]),
                        x0_dma[1])
                pp1 = constp.tile([128, 2], f32, tag=f"pp{b}1", name=f"pp{b}1")
                sop(nc.scalar.activation, pscr, i1[:, 0:2048], ACTF.Copy,
                    accum_out=pp1[:, 0:1])
                t1 = tscr[:, 2048:3072]
                vop(nc.vector.tensor_add, t1, i1[:, 2048:3072], i1[:, 3072:4096])
                vop(nc.vector.tensor_add, t1[:, 0:512], t1[:, 0:512], t1[:, 512:1024])
                vop(nc.vector.reduce_sum, pp1[:, 1:2], t1[:, 0:512], AX.X)
                pt1 = constp.tile([128, 1], f32, tag=f"pt{b}1", name=f"ptmp{b}1")
                vop(nc.vector.reduce_sum, pt1, pp1, AX.X)
                p1 = constp.tile([128, 1], bf16, tag=f"pool{b}1", name=f"pooled{b}1")
                vop(nc.vector.tensor_scalar_add, p1, pt1, 0.0)
                pooled_all[b][1] = p1

            def attn_gamma(b):
                """pooled -> gamma broadcast [128, K] in SBUF (bf16 MLP path:
                single-pass matmuls instead of fp32 LOW/HIGH pairs)."""
                hid_ps = smallps.tile([MID, 1], f32, tag="small", name=f"hid_ps{b}")
                for cb in range(2):
                    nc.tensor.matmul(
                        hid_ps,
                        fc1wT_sb[:, cb * MID : (cb + 1) * MID],
                        pooled_all[b][cb],
                        start=(cb == 0),
                        stop=(cb == 1),
                    )
                hid_sb = constp.tile([MID + 1, 1], bf16, tag=f"hid{b}", name=f"hid_sb{b}")
                vop(nc.vector.memset, hid_sb[MID : MID + 1, :], 1.0)
                vop(nc.vector.tensor_scalar,
                    hid_sb[0:MID, :], hid_ps, fc1b_sb, 0.0, op0=ALU.add, op1=ALU.max)

                lg_ps = smallps.tile([1, K], f32, tag="small", name=f"lg_ps{b}")
                nc.tensor.matmul(lg_ps, hid_sb, fc2aug_sb, start=True, stop=True)

                ex = constp.tile([1, K], f32, tag=f"ex{b}", name=f"ex{b}")
                sm = constp.tile([1, 1], f32, tag=f"sm{b}", name=f"sm{b}")
                sop(nc.scalar.activation, ex, lg_ps, ACTF.Exp, accum_out=sm)
                rc = constp.tile([1, 1], f32, tag=f"rc{b}", name=f"rc{b}")
                vop(nc.vector.reciprocal, rc, sm)
                gam = constp.tile([1, K], bf16, tag=f"at{b}", name=f"gam{b}")
                vop(nc.vector.tensor_scalar,
                    gam, ex, rc, 0.25, op0=ALU.mult, op1=ALU.subtract)
                gam_ps = smallps.tile([128, K], f32, tag="small", name=f"gam_ps{b}")
                nc.tensor.matmul(gam_ps, ones_sb, gam, start=True, stop=True)
                # SBUF copy: gpsimd STT can't read PSUM, and PSUM operands
                # force DVE 1x mode
                gam_bc = constp.tile([128, K], f32, tag=f"gbs{b}", name=f"gam_bc{b}")
                sop(nc.scalar.activation, gam_bc, gam_ps, ACTF.Copy)
                return gam_bc

            # ---------------- mixing (DVE scalar_tensor_tensor, 3 passes) ----------
            wmix = [[None, None] for _ in range(NB)]
            for b in range(NB):
                for cb in range(2):
                    wmix[b][cb] = wmixp.tile(
                        [128, NPOS * O], bf16, tag=f"wm{b}{cb}", name=f"wmix{b}{cb}"
                    )

            def mix_piece(b, gam_bc, cb, src, blk_taps, t_lo, t_hi, wm_t0, ob):
                """Mix one (tap-chunk, cb, ob-half): 3 STT passes on DVE over a
                [128, ntaps, 128] strided view.  src is [wbar|wd1|wd2|wd3]
                packed with blk_taps*O cols per block; src taps [t_lo,t_hi)
                map to wmix taps starting at wm_t0.  Mixing ob0 for all
                chunks first halves the critical path to full-rate conv;
                ob1 mixes under the running conv."""
                sv = src.rearrange("c (j t o) -> c j t o", j=4, o=O)
                osl = slice(ob * 128, (ob + 1) * 128)
                ntaps = t_hi - t_lo
                wm = wmix[b][cb].rearrange("c (t o) -> c t o", o=O)[
                    :, wm_t0 : wm_t0 + ntaps, osl
                ]
                vop(nc.vector.scalar_tensor_tensor,
                    wm, sv[:, 1, t_lo:t_hi, osl], gam_bc[:, 1:2],
                    sv[:, 0, t_lo:t_hi, osl], op0=ALU.mult, op1=ALU.add)
                for j in (2, 3):
                    vop(nc.vector.scalar_tensor_tensor,
                        wm, sv[:, j, t_lo:t_hi, osl],
                        gam_bc[:, j : j + 1], wm, op0=ALU.mult, op1=ALU.add)

            def mix_sample(b, gam_bc):
                for ob in range(2):
                    if ob == 0:
                        # tap0-cb0 alone first: unblocks the PE ~1us earlier
                        mix_piece(b, gam_bc, 0, wfirst_sb[0], 5, 0, 1, 0, 0)
                        mix_piece(b, gam_bc, 0, wfirst_sb[0], 5, 1, 5, 1, 0)
                        mix_piece(b, gam_bc, 1, wfirst_sb[1], 5, 0, 5, 0, 0)
                    else:
                        for cb in range(2):
                            mix_piece(b, gam_bc, cb, wfirst_sb[cb], 5, 0, 5, 0, ob)
                    for cb in range(2):
                        mix_piece(b, gam_bc, cb, wrest2_sb[cb], 4, 0, 4, 5, ob)
                    if ob == 0:
                        # border gathers here: corr matmuls need them right
                        # after the ob0 pieces; ob1 mixing has ~8us of slack
                        gather_borders(b)

            # ---------------- border-column gathers for corrections ----------------
            gtile = [[None, None] for _ in range(NB)]

            def gather_borders(b):
                for cb in range(2):
                    g = constp.tile([128, 132], bf16, tag=f"g{b}{cb}", name=f"g{b}{cb}")
                    vop(nc.vector.tensor_scalar_add,
                        g[:, 0:66], xpad[b][cb][:, 0 : 65 * W + 1 : W], 0.0)
                    vop(nc.vector.tensor_scalar_add,
                        g[:, 66:132], xpad[b][cb][:, ROW0 : ROW0 + 65 * W + 1 : W], 0.0)
                    gtile[b][cb] = g

            # ---------------- x[1] load + pads ------------------------------------
            x1_dmas = []

            def x1_dma(cb, qn, *gates):
                return gate(qdma(
                    qn,
                    xpad[1][cb][:, ROW0 : ROW0 + H * W],
                    x_d[1, cb * 128 : (cb + 1) * 128, :, :]
                    .rearrange("c h w -> c (h w)"),
                ), *gates)

            # ---------------- main conv ----------------
            def wsl(b, cb, pos, ob):
                off = pos * O + ob * 128
                return wmix[b][cb][:, off : off + 128]

            def emit_corr(b, ob):
                corr = corrps.tile([128, 128], f32, tag="corr", name=f"corr{b}{ob}")
                for side, dxv in ((0, 0), (1, 2)):
                    i = 0
                    for cb in range(2):
                        for dy in range(KS):
                            g0 = side * 66 + dy
                            nc.tensor.matmul(
                                corr[:, side * 64 : side * 64 + 64],
                                wsl(b, cb, dy * KS + dxv, ob),
                                gtile[b][cb][:, g0 : g0 + 64],
                                start=(i == 0),
                                stop=(i == 5),
                            )
                            i += 1
                # gpsimd (border fixes) can't read PSUM: drain to SBUF
                corr_sb = constp.tile([128, 128], f32, tag=f"cs{b}{ob}", name=f"corr_sb{b}{ob}")
                sop(nc.scalar.activation, corr_sb, corr, ACTF.Copy)
                return corr_sb

            POSCHUNK = ((0, 1, 2, 3, 4), (5, 6, 7, 8))

            def emit_conv_group(b, ob, rg, last=False):
                y0 = rg * 8
                cps = convps.tile([128, 512], f32, tag="conv", name=f"cps{b}{ob}{rg}")
                i = 0
                for pc in POSCHUNK:
                    for cb in range(2):
                        for pos in pc:
                            dy, dx = divmod(pos, KS)
                            s = (y0 + dy) * W + dx
                            nc.tensor.matmul(
                                cps,
                                wsl(b, cb, pos, ob),
                                xpad[b][cb][:, s : s + 512],
                                start=(i == 0),
                                stop=(i == 17),
                            )
                            i += 1
                osb = osbp.tile([128, 512], bf16, tag="osb", name=f"osb{b}{ob}{rg}")
                if last:
                    # split the final PSUM drain across scalar + DVE to
                    # shorten the end-of-kernel chain
                    sop(nc.scalar.activation, osb[:, 0:256], cps[:, 0:256], ACTF.Copy)
                    vop(nc.vector.tensor_scalar_add, osb[:, 256:512], cps[:, 256:512], 0.0)
                else:
                    sop(nc.scalar.activation, osb, cps, ACTF.Copy)
                return osb

            def emit_fix_and_out(b, ob, rg, osb, corr, last):
                y0 = rg * 8
                ov = osb.rearrange("m (y x) -> m y x", x=W)[:, :, 0 : W : W - 1]
                cv = corr.rearrange("m (s y) -> m y s", s=2)[:, y0 : y0 + 8, :]
                dst = out_d[b, ob * 128 : (ob + 1) * 128, y0 : y0 + 8, :]
                src = osb.rearrange("m (y x) -> m y x", x=W)
                # last section avoids the gpsimd queue so its end-of-kernel
                # drain is short
                tail = b == NB - 1
                if last:
                    # y-halved fixes (each gated on its copy half) + quartered
                    # outputs so the final drain chain is as short as possible
                    gop(nc.gpsimd.tensor_sub, ov[:, 0:4], ov[:, 0:4], cv[:, 0:4])
                    gop(nc.gpsimd.tensor_sub, ov[:, 4:8], ov[:, 4:8], cv[:, 4:8])
                    # partition halves keep 1KB runs (y-splits would be 512B
                    # runs at ~quarter DMA rate)
                    qdma("sync", dst[0:64], src[0:64])
                    qdma("scalar", dst[64:128], src[64:128])
                    return
                gop(nc.gpsimd.tensor_sub, ov, ov, cv)
                if rg % 2 == 0:
                    qdma("scalar" if tail else "gpsimd", dst, src)
                else:
                    qdma("sync", dst, src)

            def conv_section(b, ob, corr_after=2):
                pend = []
                for rg in range(corr_after):
                    pend.append((rg, emit_conv_group(b, ob, rg)))
                corr = emit_corr(b, ob)
                for rg, osb in pend:
                    emit_fix_and_out(b, ob, rg, osb, corr, last=False)
                for rg in range(corr_after, 8):
                    last = b == NB - 1 and ob == 1 and rg == 7
                    osb = emit_conv_group(b, ob, rg, last=last)
                    emit_fix_and_out(b, ob, rg, osb, corr, last=last)

            # ================= schedule =================
            for cb in range(2):
                t = xpadp.tile([128, FPAD], bf16, tag=f"xpad1{cb}", name=f"xpad1{cb}")
                gop(nc.gpsimd.memset, t[:, 0:ROW0], 0.0)
                gop(nc.gpsimd.memset, t[:, ROW0 + H * W : FPAD], 0.0)
                xpad[1][cb] = t

            pool_sample(0)
            # all four weight tensors chain on the FAST sync queue (qAct is
            # ~135 GB/s vs sync ~190); qAct carries only x0-cb1 + wfirst-cb1.
            # x1 is gated behind the whole weight chain so it never steals
            # HBM from the conv-gating transfers (it is needed ~50us later).
            wr_dma[0] = gate(qdma("sync", wrest2_sb[0], wrest2_d[rows(0), :]),
                             wf_dma[0])
            wr_dma[1] = gate(qdma("sync", wrest2_sb[1], wrest2_d[rows(1), :]),
                             wr_dma[0])
            gbc0 = attn_gamma(0)
            x1_dmas.append(x1_dma(0, "sync", wr_dma[1]))
            x1_dmas.append(x1_dma(1, "gpsimd", wr_dma[1]))
            mix_sample(0, gbc0)

            conv_section(0, 0, corr_after=3)
            # sample 1's attention overlaps sample 0's conv
            pool_sample(1)
            gbc1 = attn_gamma(1)
            mix_sample(1, gbc1)
            conv_section(0, 1)
            conv_section(1, 0)
            conv_section(1, 1)

    nc.compile()
    return nc


def get_nc():
    if "nc" not in _CACHE:
        _CACHE["nc"] = _build_nc()
    return _CACHE["nc"]


def prep_inputs(x, w_dyn, fc1_w, fc1_b, fc2_w, fc2_b):
    """Host-side layout prep + batch sharding -> per-core input maps."""
    import ml_dtypes

    bf16 = ml_dtypes.bfloat16
    wt = np.transpose(np.asarray(w_dyn, np.float32), (0, 2, 3, 4, 1)).reshape(
        K, C, NPOS * O
    )
    wbar = wt.mean(axis=0)                      # [C, 9*O]
    wd = wt[1:] - wt[0:1]                       # [3, C, 9*O]
    srcs = [wbar, wd[0], wd[1], wd[2]]
    wfirst = np.concatenate([s[:, 0 : 5 * O] for s in srcs], axis=1).astype(bf16)
    wrest2 = np.concatenate([s[:, 5 * O : 9 * O] for s in srcs], axis=1).astype(bf16)
    fc1wT = (np.ascontiguousarray(np.asarray(fc1_w, np.float32).T) / float(H * W)).astype(bf16)
    fc1b = np.ascontiguousarray(np.asarray(fc1_b, np.float32).reshape(MID, 1))
    fc2aug = np.ascontiguousarray(
        np.vstack([np.asarray(fc2_w, np.float32).T, np.asarray(fc2_b, np.float32)[None, :]])
        * INV_DELTA
    ).astype(bf16)
    x = np.asarray(x, np.float32).astype(bf16)
    in_maps = []
    for core in range(NCORES):
        in_maps.append(
            {
                "x": np.ascontiguousarray(x[core * NB : (core + 1) * NB]),
                "wfirst": np.ascontiguousarray(wfirst),
                "wrest2": np.ascontiguousarray(wrest2),
                "fc1wT": fc1wT,
                "fc1b": fc1b,
                "fc2aug": fc2aug,
            }
        )
    return in_maps


def kernel(x, w_dyn, fc1_w, fc1_b, fc2_w, fc2_b):
    from concourse.bass_utils import run_bass_kernel_spmd

    nc = get_nc()
    in_maps = prep_inputs(x, w_dyn, fc1_w, fc1_b, fc2_w, fc2_b)
    res = run_bass_kernel_spmd(nc, in_maps, core_ids=list(range(NCORES)))
    return np.concatenate(
        [r["out"].astype(np.float32) for r in res.results], axis=0
    )
